# revision 32
# baseline (speedup 1.0000x reference)
"""EMA recurrence kernel for Trainium2 (8 NeuronCores, Bass/Tile).

Computes a_t = w * x_t + (1 - w) * a_{t-1} over inputs [B=32, T=8192, C=128],
initial_state [B, C], weights [C] -> output [B, T, C].

Strategy (fp16 streaming, no on-device transposes):
  - Pure data parallelism: batch dim sharded 4-per-core across 8 cores.
  - Host pre-shards to [BL, C, T] fp16 (channel-major), so the device sees
    channels on SBUF partitions directly; time is the free dim the DVE scan
    runs along. Host post-gathers [BL, C, T] fp16 -> [B, T, C] fp32.
  - fp16 I/O halves HBM traffic vs fp32 (memory-bound problem); the scan
    recurrence state is fp32 in hardware regardless of operand dtype, so
    the dominant precision loss is the fp16 rounding of the decay
    coefficients (~8e-3 rel, vs the 2e-2 harness gate).
  - Per core, per (batch, 2048-step chunk):
      * DMA in on the SP HWDGE ring ([128, 2048] fp16, 4KB/partition runs)
      * ACT: xw = w * x via per-partition activation scale (fp16 out),
        into a [C, 1+2048] tile whose column 0 holds the carry
      * DVE tensor_tensor_scan over [C, 2049]: a_t = (1-w)*a_{t-1} + xw_t
        along time; chunks chain through the carry column, not the scan's
        initial operand
      * DMA out (cols 1..2048) on the Pool SWDGE ring, keeping the ACT and
        SP sequencer streams free of scan-dependent head-of-line blocking
  - DVE tensor_tensor_scan measures ~2.1 ns/elem on operands freshly
    produced by DMA/ACT (any size/dtype/initial tried); the kernel is that
    scan-throughput-bound, with DMA/ACT/Pool fully overlapped under it.
"""

import sys

if "/opt/trn_rl_repo" not in sys.path:
    sys.path.insert(0, "/opt/trn_rl_repo")

import numpy as np

B, T, C = 32, 8192, 128
NCORES = 8
BL = B // NCORES      # batches per core (4)
CHUNK = 2048          # time steps per scan chunk
# chunk schedule per batch: the last 2048 steps run as two 1024-step chunks
# so the final out-DMA chain starts ~2us earlier (shorter tail); cold-scan
# cost is ~flat per element, so DVE total is unchanged
CHUNKS = [(0, 2048), (2048, 2048), (4096, 2048), (6144, 1024), (7168, 1024)]

_NC_CACHE = None


def build_bass():
    global _NC_CACHE
    if _NC_CACHE is not None:
        return _NC_CACHE

    import concourse.bacc as bacc
    import concourse.mybir as mybir
    import concourse.tile as tile

    f32 = mybir.dt.float32
    f16 = mybir.dt.float16
    AF = mybir.ActivationFunctionType
    ALU = mybir.AluOpType

    nc = bacc.Bacc("TRN2", target_bir_lowering=False, debug=False)
    x = nc.dram_tensor("x", [BL, C, T], f16, kind="ExternalInput").ap()
    s0T = nc.dram_tensor("s0T", [C, BL], f32, kind="ExternalInput").ap()
    wcol = nc.dram_tensor("wcol", [C, 1], f32, kind="ExternalInput").ap()
    y = nc.dram_tensor("y", [BL, C, T], f16, kind="ExternalOutput").ap()

    with tile.TileContext(nc) as tc:
        with (
            tc.tile_pool(name="const", bufs=1) as cpool,
            tc.tile_pool(name="xin", bufs=3) as xpool,
            tc.tile_pool(name="xw", bufs=3) as wpool,
            tc.tile_pool(name="yo", bufs=3) as ypool,
        ):
            wcol_t = cpool.tile([C, 1], f32, name="wcol_t")
            nc.scalar.dma_start(wcol_t[:], wcol[:])
            s0T_t = cpool.tile([C, BL], f32, name="s0T_t")
            nc.scalar.dma_start(s0T_t[:], s0T[:])
            # cdec = 1 - w, materialized on device in fp16.
            # CHUNK+1 wide: column 0 pairs with the carry column in xw.
            cdec_t = cpool.tile([C, CHUNK + 1], f16, name="cdec_t")
            nc.scalar.activation(
                cdec_t[:],
                wcol_t[:, 0:1].to_broadcast((C, CHUNK + 1)),
                AF.Copy,
                scale=-1.0,
                bias=1.0,
            )

            prev = {}       # (tile, length) of previous chunk per batch
            for k, (t0, ln) in enumerate(CHUNKS):
                for b in range(BL):
                    nb = 3 if ln == CHUNK else 2
                    xt = xpool.tile(
                        [C, ln], f16, name=f"xt{b}_{k}", tag=f"xt{b}_{ln}", bufs=nb
                    )
                    # first chunk: DMA and prescale in two halves so the
                    # first scan's inputs are ready ~a half-chunk earlier
                    # (startup is serial: preamble -> DMA -> ACT -> scan)
                    # round 0 loads ride the scalar HWDGE ring, which starts
                    # moving data ~5us before the sync ring does (measured);
                    # steady-state input stays on the otherwise-idle sync ring
                    nparts = 2 if k == 0 else 1
                    hl = ln // nparts
                    ring = nc.scalar if k == 0 else nc.sync
                    for p in range(nparts):
                        ring.dma_start(
                            xt[:, p * hl : (p + 1) * hl],
                            x[b][:, t0 + p * hl : t0 + (p + 1) * hl],
                        )
                    # xw has a leading carry column: xw[:,0] = a[last of prev
                    # chunk], so the scan runs with an immediate initial
                    # (state_0 = (c*0) + carry) and chunks chain through
                    # data instead of the initial operand.
                    xw = wpool.tile(
                        [C, ln + 1], f16, name=f"xw{b}_{k}", tag=f"xw{b}_{ln}",
                        bufs=nb,
                    )
                    if k == 0:
                        carry = s0T_t[:, b : b + 1]
                    else:
                        pt, pl = prev[b]
                        carry = pt[:, pl : pl + 1]
                    nc.scalar.activation(xw[:, 0:1], carry, AF.Copy)
                    for p in range(nparts):
                        nc.scalar.activation(
                            xw[:, 1 + p * hl : 1 + (p + 1) * hl],
                            xt[:, p * hl : (p + 1) * hl],
                            AF.Copy,
                            scale=wcol_t[:],
                        )
                    yt = ypool.tile(
                        [C, ln + 1], f16, name=f"yt{b}_{k}", tag=f"yt{b}_{ln}",
                        bufs=nb,
                    )
                    # col 0: state becomes (c*0)*... + carry = carry; real
                    # outputs land in cols 1..ln; col 0 is discarded.
                    nc.vector.tensor_tensor_scan(
                        yt[:], cdec_t[:, 0 : ln + 1], xw[:], 0.0,
                        op0=ALU.mult, op1=ALU.add,
                    )
                    prev[b] = (yt, ln)
                    # out-DMA via Pool SWDGE: keeps the ACT sequencer stream
                    # pure activations (no head-of-line blocking on scan deps)
                    nc.gpsimd.dma_start(
                        y[b][:, t0 : t0 + ln], yt[:, 1 : ln + 1]
                    )

    nc.compile()
    _NC_CACHE = nc
    return nc


def _in_maps(inputs, initial_state, weights):
    x = np.asarray(inputs, dtype=np.float32)
    s0 = np.asarray(initial_state, dtype=np.float32)
    w = np.clip(np.asarray(weights, dtype=np.float32), 0.0, 1.0)
    wcol = np.ascontiguousarray(w[:, None])

    xT = x.astype(np.float16).transpose(0, 2, 1)  # [B, C, T] view
    maps = []
    for i in range(NCORES):
        maps.append(
            {
                "x": np.ascontiguousarray(xT[i * BL : (i + 1) * BL]),
                "s0T": np.ascontiguousarray(s0[i * BL : (i + 1) * BL].T),
                "wcol": wcol,
            }
        )
    return maps


def _ensure_ntff_hook():
    """Shim antenv.axon_hooks (absent in this image) so trace=True works."""
    import types

    import antenv

    if not hasattr(antenv, "axon_hooks"):
        mod = types.ModuleType("antenv.axon_hooks")
        holder = [None]
        mod.set_axon_ntff_profile_hook = lambda h: holder.__setitem__(0, h)
        mod.get_axon_ntff_profile_hook = lambda: holder[0]
        sys.modules["antenv.axon_hooks"] = mod
        antenv.axon_hooks = mod
    from antenv.axon_hooks import (
        get_axon_ntff_profile_hook,
        set_axon_ntff_profile_hook,
    )

    if get_axon_ntff_profile_hook() is None:
        from trn_agent_boot.trn_boot import _ntff_profile_via_ctypes

        set_axon_ntff_profile_hook(
            _ntff_profile_via_ctypes("/opt/axon/libaxon_pjrt.so")
        )


def run(inputs, initial_state, weights, trace=False, **kw):
    from concourse import bass_utils

    if trace:
        _ensure_ntff_hook()
    nc = build_bass()
    maps = _in_maps(inputs, initial_state, weights)
    res = bass_utils.run_bass_kernel_spmd(
        nc, maps, core_ids=list(range(NCORES)), trace=trace, **kw
    )
    yT = np.concatenate([r["y"] for r in res.results], axis=0)  # [B, C, T] fp16
    out = yT.transpose(0, 2, 1).astype(np.float32)
    return out, res


def kernel(inputs, initial_state, weights):
    out, _ = run(inputs, initial_state, weights)
    return out


# revision 33
# speedup vs baseline: 1.0911x; 1.0911x over previous
"""EMA recurrence kernel for Trainium2 (8 NeuronCores, Bass/Tile).

Computes a_t = w * x_t + (1 - w) * a_{t-1} over inputs [B=32, T=8192, C=128],
initial_state [B, C], weights [C] -> output [B, T, C].

Strategy (fp16 streaming, no on-device transposes):
  - Pure data parallelism: batch dim sharded 4-per-core across 8 cores.
  - Host pre-shards to [BL, C, T] fp16 (channel-major), so the device sees
    channels on SBUF partitions directly; time is the free dim the DVE scan
    runs along. Host post-gathers [BL, C, T] fp16 -> [B, T, C] fp32.
  - fp16 I/O halves HBM traffic vs fp32 (memory-bound problem); the scan
    recurrence state is fp32 in hardware regardless of operand dtype, so
    the dominant precision loss is the fp16 rounding of the decay
    coefficients (~8e-3 rel, vs the 2e-2 harness gate).
  - Per core, per (batch, 2048-step chunk):
      * DMA in on the SP HWDGE ring ([128, 2048] fp16, 4KB/partition runs)
      * ACT: xw = w * x via per-partition activation scale (fp16 out),
        into a [C, 1+2048] tile whose column 0 holds the carry
      * DVE tensor_tensor_scan over [C, 2049]: a_t = (1-w)*a_{t-1} + xw_t
        along time; chunks chain through the carry column, not the scan's
        initial operand
      * DMA out (cols 1..2048) on the Pool SWDGE ring, keeping the ACT and
        SP sequencer streams free of scan-dependent head-of-line blocking
  - DVE tensor_tensor_scan measures ~2.1 ns/elem on operands freshly
    produced by DMA/ACT (any size/dtype/initial tried); the kernel is that
    scan-throughput-bound, with DMA/ACT/Pool fully overlapped under it.
"""

import sys

if "/opt/trn_rl_repo" not in sys.path:
    sys.path.insert(0, "/opt/trn_rl_repo")

import numpy as np

B, T, C = 32, 8192, 128
NCORES = 8
BL = B // NCORES      # batches per core (4)
CHUNK = 2048          # time steps per scan chunk
# chunk schedule per batch: the last 2048 steps run as two 1024-step chunks
# so the final out-DMA chain starts ~2us earlier (shorter tail); cold-scan
# cost is ~flat per element, so DVE total is unchanged
CHUNKS = [(0, 2048), (2048, 2048), (4096, 2048), (6144, 1024), (7168, 1024)]

_NC_CACHE = None


def build_bass():
    global _NC_CACHE
    if _NC_CACHE is not None:
        return _NC_CACHE

    import concourse.bacc as bacc
    import concourse.mybir as mybir
    import concourse.tile as tile

    f32 = mybir.dt.float32
    f16 = mybir.dt.float16
    AF = mybir.ActivationFunctionType
    ALU = mybir.AluOpType

    nc = bacc.Bacc("TRN2", target_bir_lowering=False, debug=False)
    x = nc.dram_tensor("x", [BL, C, T], f16, kind="ExternalInput").ap()
    s0T = nc.dram_tensor("s0T", [C, BL], f32, kind="ExternalInput").ap()
    wcol = nc.dram_tensor("wcol", [C, 1], f32, kind="ExternalInput").ap()
    y = nc.dram_tensor("y", [BL, C, T], f16, kind="ExternalOutput").ap()

    with tile.TileContext(nc) as tc:
        with (
            tc.tile_pool(name="const", bufs=1) as cpool,
            tc.tile_pool(name="xin", bufs=3) as xpool,
            tc.tile_pool(name="xw", bufs=3) as wpool,
            tc.tile_pool(name="yo", bufs=3) as ypool,
        ):
            wcol_t = cpool.tile([C, 1], f32, name="wcol_t")
            nc.scalar.dma_start(wcol_t[:], wcol[:])
            s0T_t = cpool.tile([C, BL], f32, name="s0T_t")
            nc.scalar.dma_start(s0T_t[:], s0T[:])
            # cdec = 1 - w, materialized on device in fp16.
            # CHUNK+1 wide: column 0 pairs with the carry column in xw.
            cdec_t = cpool.tile([C, CHUNK + 1], f16, name="cdec_t")
            nc.scalar.activation(
                cdec_t[:],
                wcol_t[:, 0:1].to_broadcast((C, CHUNK + 1)),
                AF.Copy,
                scale=-1.0,
                bias=1.0,
            )

            prev = {}       # (tile, length) of previous chunk per batch
            for k, (t0, ln) in enumerate(CHUNKS):
                for b in range(BL):
                    nb = 3 if ln == CHUNK else 2
                    xt = xpool.tile(
                        [C, ln], f16, name=f"xt{b}_{k}", tag=f"xt{b}_{ln}", bufs=nb
                    )
                    # first chunk: DMA and prescale in two halves so the
                    # first scan's inputs are ready ~a half-chunk earlier
                    # (startup is serial: preamble -> DMA -> ACT -> scan)
                    nparts = 2 if k == 0 else 1
                    hl = ln // nparts
                    for p in range(nparts):
                        nc.sync.dma_start(
                            xt[:, p * hl : (p + 1) * hl],
                            x[b][:, t0 + p * hl : t0 + (p + 1) * hl],
                        )
                    # xw has a leading carry column: xw[:,0] = a[last of prev
                    # chunk], so the scan runs with an immediate initial
                    # (state_0 = (c*0) + carry) and chunks chain through
                    # data instead of the initial operand.
                    xw = wpool.tile(
                        [C, ln + 1], f16, name=f"xw{b}_{k}", tag=f"xw{b}_{ln}",
                        bufs=nb,
                    )
                    if k == 0:
                        carry = s0T_t[:, b : b + 1]
                    else:
                        pt, pl = prev[b]
                        carry = pt[:, pl : pl + 1]
                    nc.scalar.activation(xw[:, 0:1], carry, AF.Copy)
                    for p in range(nparts):
                        nc.scalar.activation(
                            xw[:, 1 + p * hl : 1 + (p + 1) * hl],
                            xt[:, p * hl : (p + 1) * hl],
                            AF.Copy,
                            scale=wcol_t[:],
                        )
                    yt = ypool.tile(
                        [C, ln + 1], f16, name=f"yt{b}_{k}", tag=f"yt{b}_{ln}",
                        bufs=nb,
                    )
                    # col 0: state becomes (c*0)*... + carry = carry; real
                    # outputs land in cols 1..ln; col 0 is discarded.
                    nc.vector.tensor_tensor_scan(
                        yt[:], cdec_t[:, 0 : ln + 1], xw[:], 0.0,
                        op0=ALU.mult, op1=ALU.add,
                    )
                    prev[b] = (yt, ln)
                    # out-DMA via Pool SWDGE: keeps the ACT sequencer stream
                    # pure activations (no head-of-line blocking on scan deps)
                    nc.gpsimd.dma_start(
                        y[b][:, t0 : t0 + ln], yt[:, 1 : ln + 1]
                    )

    nc.compile()
    _NC_CACHE = nc
    return nc


def _in_maps(inputs, initial_state, weights):
    x = np.asarray(inputs, dtype=np.float32)
    s0 = np.asarray(initial_state, dtype=np.float32)
    w = np.clip(np.asarray(weights, dtype=np.float32), 0.0, 1.0)
    wcol = np.ascontiguousarray(w[:, None])

    xT = x.astype(np.float16).transpose(0, 2, 1)  # [B, C, T] view
    maps = []
    for i in range(NCORES):
        maps.append(
            {
                "x": np.ascontiguousarray(xT[i * BL : (i + 1) * BL]),
                "s0T": np.ascontiguousarray(s0[i * BL : (i + 1) * BL].T),
                "wcol": wcol,
            }
        )
    return maps


def _ensure_ntff_hook():
    """Shim antenv.axon_hooks (absent in this image) so trace=True works."""
    import types

    import antenv

    if not hasattr(antenv, "axon_hooks"):
        mod = types.ModuleType("antenv.axon_hooks")
        holder = [None]
        mod.set_axon_ntff_profile_hook = lambda h: holder.__setitem__(0, h)
        mod.get_axon_ntff_profile_hook = lambda: holder[0]
        sys.modules["antenv.axon_hooks"] = mod
        antenv.axon_hooks = mod
    from antenv.axon_hooks import (
        get_axon_ntff_profile_hook,
        set_axon_ntff_profile_hook,
    )

    if get_axon_ntff_profile_hook() is None:
        from trn_agent_boot.trn_boot import _ntff_profile_via_ctypes

        set_axon_ntff_profile_hook(
            _ntff_profile_via_ctypes("/opt/axon/libaxon_pjrt.so")
        )


def run(inputs, initial_state, weights, trace=False, **kw):
    from concourse import bass_utils

    if trace:
        _ensure_ntff_hook()
    nc = build_bass()
    maps = _in_maps(inputs, initial_state, weights)
    res = bass_utils.run_bass_kernel_spmd(
        nc, maps, core_ids=list(range(NCORES)), trace=trace, **kw
    )
    yT = np.concatenate([r["y"] for r in res.results], axis=0)  # [B, C, T] fp16
    out = yT.transpose(0, 2, 1).astype(np.float32)
    return out, res


def kernel(inputs, initial_state, weights):
    out, _ = run(inputs, initial_state, weights)
    return out
